# revision 55
# baseline (speedup 1.0000x reference)
"""BertLayer (attention + adapter + FFN + LayerNorm) Trainium2 Bass kernel, v2.

Sharding: 8 cores, pure SPMD (no collectives). Core c handles batch b=c//4
and query rows [q0, q0+512) with q0=(c%4)*512. Each core computes K/V for
its full batch locally (replicated within the 4-core batch group), then
attention / adapter / FFN / LayerNorm for its 512 rows.

All on-chip compute is done in the "transposed" orientation (feature dim
on partitions, token dim on the free axis) so that every matmul has its
contraction dim on partitions and no on-device transposes are needed.

v2 over the original kernel:
- The K-projection bias is dropped entirely: softmax over kpos is invariant
  to per-query offsets, and (q+bq).(k+bk) differs from (q+bq).k only by
  f(q) terms, so bk cancels exactly in the softmax normalization.
- Q-bias and K/V PSUM evictions run on VectorE; ScalarE keeps only
  exp / gelu / a few copies (it was the attention-phase bottleneck).
- Attention runs in 3 "waves" of 2 head-pairs. A wave's K/V projection
  (TensorE-heavy, ScalarE-idle) interleaves with its own scores/context
  (ScalarE-heavy), keeping both engines dense.
- A head pair's two score matmuls write the two banks of one PSUM tile:
  even head rows 0-63, odd head rows 64-127 (disjoint PE row-groups ->
  the HW overlaps them), followed by ONE merged exp over 1024 columns
  (valid because the additive mask is zero -> uniform exp bias; a nonzero
  mask compiles a fallback variant with per-kpos-tile biased exps).
- The softmax denominator comes from a ones-column appended to V in the
  context matmul; V bias is applied after normalization.

Matmul operands are fp16 (1 cycle/row on the PE); accumulation stays fp32
in PSUM. The exp bias of -2 guards fp16 exp overflow and cancels in the
softmax normalization.
"""

import numpy as np

import concourse.bass as bass
import concourse.mybir as mybir
import concourse.tile as tile
from concourse import bacc
from concourse.bass_utils import run_bass_kernel_spmd
from contextlib import ExitStack

F32 = mybir.dt.float32
F16 = mybir.dt.float16
F8 = mybir.dt.float8e4
AF = mybir.ActivationFunctionType
DR = mybir.MatmulPerfMode.DoubleRow
WI_SCALE = 16.0  # lifts wi (std 0.02) out of fp8-e4m3's subnormal range

B, S, H = 2, 2048, 768
NH, DH = 12, 64
FF = 3072
AD = 64
EPS = 1e-12
P = 128
KO = H // P          # 6 partition-tiles of the hidden dim
Q = 512              # query rows per core
NCORES = 8
NCH = 4              # kpos chunks (512 each)
CH = S // NCH        # 512
JT = CH // P         # 4 kpos 128-tiles per chunk
FFT = FF // P        # 24
VH = AD + 1          # per-head V columns incl. ones column
NW = 3               # attention waves (2 head-pairs each)
CC = 77              # consts columns


def r(ap):
    return ap


def _build_nc(uniform_mask: bool):
    nc = bacc.Bacc(
        "TRN2",
        target_bir_lowering=False,
        debug=False,
        num_devices=NCORES,
    )

    def din(name, shape, dt=F32):
        return nc.dram_tensor(name, list(shape), dt, kind="ExternalInput").ap()

    xt = din("xt", (H, S), F16)        # hidden[b].T
    xtq = din("xtq", (H, Q), F16)      # hidden[b, q0:q0+Q].T
    wqt = din("wqt", (H, H), F16)
    wkt = din("wkt", (H, H), F16)
    wvt = din("wvt", (H, H), F16)
    afit = din("afit", (H, AD), F16)
    aset = din("aset", (AD, H), F16)
    wit = din("wit", (H, FF), F16)
    wot = din("wot", (FF, H), F16)
    consts = din("consts", (P, CC))
    outt = nc.dram_tensor("outt", [H, Q], F16, kind="ExternalOutput").ap()

    def part6(ap):  # [(ko p), n] -> [p, ko, n]
        return ap.rearrange("(ko p) n -> p ko n", p=P)

    with tile.TileContext(nc) as tc, nc.allow_low_precision(
        reason="fp16 matmul operands; accumulation stays fp32 in PSUM"
    ), ExitStack() as top:
        const = top.enter_context(tc.tile_pool(name="const", bufs=1))
        persist = top.enter_context(tc.tile_pool(name="persist", bufs=1))
        aw_pool = top.enter_context(tc.tile_pool(name="aw", bufs=1))

        consts_sb = const.tile([P, CC], F32, tag="consts")
        maskt_sb = consts_sb[:, 0:16]
        bq_sb = consts_sb[:, 16:22]
        bv_sb = consts_sb[:, 22:28]
        aseb_sb = consts_sb[:, 28:34]
        bo_sb = consts_sb[:, 34:40]
        lng_sb = consts_sb[:, 40:46]
        lnb_sb = consts_sb[:, 46:52]
        bi_sb = consts_sb[:, 52:76]
        afib_sb = consts_sb[0:AD, 76:77]
        ones_col = const.tile([P, 1], F16, tag="ones")
        nc.vector.memset(ones_col[:], 1.0)
        ones_row = const.tile([1, P], F16, tag="ones_row")
        nc.vector.memset(ones_row[:], 1.0)
        # pair-broadcast selector: bc = ind2.T @ rc replicates rc row 0 over
        # partitions 0-63 and row 1 over partitions 64-127
        neg2 = const.tile([P, 1], F32, tag="neg2")
        nc.vector.memset(neg2[:], -2.0)
        one_sb = const.tile([1, 1], F32, tag="one_sb")
        nc.vector.memset(one_sb[:], 1.0)
        ones_q = const.tile([1, Q], F16, tag="ones_q")
        nc.vector.memset(ones_q[:], 1.0)
        junk = const.tile([1, 1], F32, tag="junk")
        # dummy activation: pulls the exp table load off the critical path
        # (overlaps the initial weight DMAs)
        nc.scalar.activation(junk[:], ones_col[0:1, 0:1], AF.Exp)

        # normalized attention output, transposed [H, Q], one tile per
        # 128-row block (= head pair) so downstream contractions start
        # per-block
        aot = [
            persist.tile([P, Q], F16, tag=f"aot{m}", name=f"aot{m}")
            for m in range(KO)
        ]

        # ================= attention ======================================
        with ExitStack() as s01:
            xt_pool = s01.enter_context(tc.tile_pool(name="xt", bufs=1))
            qt_pool = s01.enter_context(tc.tile_pool(name="qt", bufs=1))
            QT = qt_pool.tile([P, KO, Q], F16, tag="QT")

            # --- stage 0: QT = wq-contracted projection of the query slice
            # w0_pool lives at s01 scope: if it were freed right after the
            # Q projection, the wk/wv tiles would reuse its SBUF region and
            # their DMAs would stall on a WAR hazard until the last Q-proj
            # matmul has read the wq tiles (costs ~6us of SP idle)
            w0_pool = s01.enter_context(tc.tile_pool(name="w0", bufs=1))
            with ExitStack() as s0:
                p0_pool = s0.enter_context(
                    tc.tile_pool(name="p0", bufs=2, space="PSUM")
                )
                wqc, xtqc = [], []
                for k in range(KO):
                    wt = w0_pool.tile([P, H], F16, tag=f"wq{k}", name=f"wq{k}")
                    nc.sync.dma_start(wt[:], wqt[k * P:(k + 1) * P, :])
                    wqc.append(wt)
                    xq = w0_pool.tile([P, Q], F16, tag=f"xtq{k}", name=f"xtq{k}")
                    nc.gpsimd.dma_start(xq[:], xtq[k * P:(k + 1) * P, :])
                    xtqc.append(xq)
                nc.gpsimd.dma_start(consts_sb[:], consts)
                for ko in range(KO):
                    qp = p0_pool.tile([P, Q], F32, tag="qp")
                    for k in range(KO):
                        nc.tensor.matmul(
                            qp[:],
                            r(wqc[k][:, ko * P:(ko + 1) * P]),
                            r(xtqc[k][:]),
                            start=(k == 0),
                            stop=(k == KO - 1),
                        )
                    nc.vector.tensor_scalar_add(
                        QT[:, ko, :], qp[:], bq_sb[:, ko:ko + 1]
                    )

            # weights + hidden chunks resident for all waves
            wk_pool = s01.enter_context(tc.tile_pool(name="wk", bufs=1))
            wv_pool = s01.enter_context(tc.tile_pool(name="wv", bufs=1))
            wkc, wvc, xtc = [], [], []
            xtp = part6(xt)
            for k in range(KO):
                wt = wk_pool.tile([P, H], F16, tag=f"wk{k}", name=f"wk{k}")
                nc.sync.dma_start(wt[:], wkt[k * P:(k + 1) * P, :])
                wkc.append(wt)
            # xt chunks ride the gpsimd (SWDGE) DMA queue so they don't
            # serialize behind the weight DMAs on the sync queue
            t = xt_pool.tile([P, KO, CH], F16, tag="xt0", name="xt0")
            nc.gpsimd.dma_start(t[:], xtp[:, :, 0:CH])
            xtc.append(t)
            for k in range(KO):
                wt = wv_pool.tile([P, H], F16, tag=f"wv{k}", name=f"wv{k}")
                nc.sync.dma_start(wt[:], wvt[k * P:(k + 1) * P, :])
                wvc.append(wt)
            for c in range(1, NCH):
                t = xt_pool.tile([P, KO, CH], F16, tag=f"xt{c}", name=f"xt{c}")
                nc.gpsimd.dma_start(t[:], xtp[:, :, c * CH:(c + 1) * CH])
                xtc.append(t)
            # prefetch adapter weights (tiny) so stage 2 doesn't wait on them
            afit_sb = aw_pool.tile([P, KO, AD], F16, tag="afit")
            nc.sync.dma_start(afit_sb[:], part6(afit))
            aset_sb = aw_pool.tile([AD, H], F16, tag="aset")
            nc.sync.dma_start(aset_sb[:], aset)

            kt_pool = s01.enter_context(tc.tile_pool(name="kt", bufs=2))
            vp_pool = s01.enter_context(tc.tile_pool(name="vp", bufs=2))
            et_pool = s01.enter_context(tc.tile_pool(name="et", bufs=6))
            nrm_pool = s01.enter_context(tc.tile_pool(name="nrm", bufs=2))
            mmp = s01.enter_context(tc.tile_pool(name="mmp", bufs=2, space="PSUM"))
            cxp = s01.enter_context(tc.tile_pool(name="cxp", bufs=4, space="PSUM"))

            def emit_ctx(w, c, j, ets, cps=None, ktw=None, vpw=None):
                cps = cps or cur_cps[0]
                vpw = vpw or cur_vpw[0]
                jj = c * JT + j
                for ff in range(2):
                    f = 2 * w + ff
                    et = ets[ff]
                    for half in range(2):
                        nc.tensor.matmul(
                            cps[2 * ff + half][:],
                            r(vpw[c][:, j, 2 * ff + half, :]),
                            r(et[:, half * Q:(half + 1) * Q]),
                            start=(jj == 0),
                            stop=(jj == NCH * JT - 1),
                        )
                    if jj == NCH * JT - 1:
                        # normalize this pair right after its last context
                        # matmul (overlaps the other pair's remaining work)
                        cp0, cp1 = cps[2 * ff], cps[2 * ff + 1]
                        rc0 = nrm_pool.tile([1, Q], F16, tag="rc0")
                        rc1 = nrm_pool.tile([1, Q], F16, tag="rc1")
                        nc.vector.reciprocal(rc0[:], cp0[AD:VH, :])
                        nc.vector.reciprocal(rc1[:], cp1[AD:VH, :])
                        # replicate 1/denom over the pair's partition halves
                        # via rank-1 matmuls
                        bc = mmp.tile([P, Q], F32, tag="mm")
                        nc.tensor.matmul(
                            bc[0:DH, :], r(ones_row[:, 0:DH]),
                            r(rc0[:]), start=True, stop=True,
                        )
                        nc.tensor.matmul(
                            bc[DH:P, :], r(ones_row[:, 0:DH]),
                            r(rc1[:]), start=True, stop=True,
                        )
                        bcs = nrm_pool.tile([P, Q], F16, tag="bcs")
                        nc.scalar.activation(bcs[:], bc[:], AF.Copy)
                        nc.vector.tensor_mul(
                            aot[f][0:DH, :], cp0[0:AD, :], bcs[0:DH, :]
                        )
                        nc.vector.tensor_mul(
                            aot[f][DH:P, :], cp1[0:AD, :], bcs[DH:P, :]
                        )
                        nc.vector.tensor_scalar_add(
                            aot[f][:], aot[f][:], bv_sb[:, f:f + 1]
                        )

            pending = None
            cur_cps = [None]
            cur_vpw = [None]
            for w in range(NW):
                h0 = 4 * w           # first head of the wave
                # K^T / V tiles for this wave's 4 heads, per chunk
                ktw = [
                    kt_pool.tile([P, 2, CH], F16, tag=f"kt{c}", name=f"kt{w}_{c}")
                    for c in range(NCH)
                ]
                vpw = [
                    vp_pool.tile([P, JT, 4, VH], F16, tag=f"vp{c}", name=f"vp{w}_{c}")
                    for c in range(NCH)
                ]
                cps = [
                    cxp.tile([VH, Q], F32, tag="cx", name=f"cx{w}_{hh}")
                    for hh in range(4)
                ]
                cur_cps[0] = cps
                cur_vpw[0] = vpw
                for c in range(NCH):
                    # K projection for pairs 2w, 2w+1 (ko tiles f=2w+ff)
                    for ff in range(2):
                        f = 2 * w + ff
                        kp = mmp.tile([P, Q], F32, tag="mm")
                        for k in range(KO):
                            nc.tensor.matmul(
                                kp[:],
                                r(wkc[k][:, f * P:(f + 1) * P]),
                                r(xtc[c][:, k, :]),
                                start=(k == 0),
                                stop=(k == KO - 1),
                            )
                        # ScalarE is idle at wave starts (no exps pending);
                        # keeps the eviction off VectorE's norm-work backlog
                        nc.scalar.activation(ktw[c][:, ff, :], kp[:], AF.Copy)
                    # V projection for the wave's 4 heads
                    nc.vector.memset(vpw[c][:, :, :, AD], 1.0)
                    for j in range(JT):
                        vq = mmp.tile([P, 4 * AD], F32, tag="mm")
                        for k in range(KO):
                            nc.tensor.matmul(
                                vq[:],
                                r(xtc[c][:, k, j * P:(j + 1) * P]),
                                r(wvc[k][:, h0 * AD:(h0 + 4) * AD]),
                                start=(k == 0),
                                stop=(k == KO - 1),
                            )
                        nc.vector.tensor_copy(
                            vpw[c][:, j, :, 0:AD],
                            vq[:].rearrange("p (h d) -> p h d", d=AD),
                        )
                    # scores + exp + context, software-pipelined: each
                    # kpos-tile's context matmuls are delayed one tile (and
                    # across chunk boundaries, past the next chunk's K/V
                    # projection) so every exp hides behind PE work
                    for j in range(JT):
                        jj = c * JT + j
                        ets = []
                        for ff in range(2):
                            f = 2 * w + ff
                            sp = mmp.tile([P, 2 * Q], F32, tag="mm")
                            for half in range(2):
                                nc.tensor.matmul(
                                    sp[:, half * Q:(half + 1) * Q],
                                    r(ktw[c][half * DH:(half + 1) * DH, ff,
                                             j * P:(j + 1) * P]),
                                    r(QT[half * DH:(half + 1) * DH, f, :]),
                                    start=True,
                                    stop=True,
                                )
                            et = et_pool.tile([P, 2 * Q], F16, tag="et")
                            if uniform_mask:
                                nc.scalar.activation(
                                    et[:], sp[:], AF.Exp, bias=neg2[:], scale=0.125
                                )
                            else:
                                for half in range(2):
                                    nc.scalar.activation(
                                        et[:, half * Q:(half + 1) * Q],
                                        sp[:, half * Q:(half + 1) * Q],
                                        AF.Exp,
                                        bias=maskt_sb[:, jj:jj + 1],
                                        scale=0.125,
                                    )
                            ets.append(et)
                        if pending is not None:
                            emit_ctx(*pending)
                        pending = (w, c, j, ets)
                        if c == NCH - 1 and j == JT - 1:
                            emit_ctx(*pending)
                            pending = None

        # ================= adapter + FFN + LayerNorm ======================
        with ExitStack() as s23:
            small = s23.enter_context(tc.tile_pool(name="small", bufs=1))
            big23 = s23.enter_context(tc.tile_pool(name="big23", bufs=1))
            mid_pool = s23.enter_context(tc.tile_pool(name="mid", bufs=6))
            ps = s23.enter_context(tc.tile_pool(name="ps", bufs=2, space="PSUM"))

            # prefetch the gelu table while the adapter down-projection runs;
            # reading aot[5] (written by the last wave's normalization) keeps
            # this AFTER every exp so the exp table isn't evicted mid-attention
            junk2 = small.tile([1, 1], F32, tag="junk2")
            nc.scalar.activation(junk2[:], aot[KO - 1][0:1, 0:1], AF.Gelu)

            # adapter down-projection + gelu -> aT [AD, Q]
            ap_ps = ps.tile([AD, Q], F32, tag="ps512")
            for k in range(KO):
                nc.tensor.matmul(
                    ap_ps[:],
                    r(afit_sb[:, k, :]),
                    r(aot[k][:]),
                    start=(k == 0),
                    stop=(k == KO - 1),
                )
            aT = mid_pool.tile([AD, Q], F16, tag="aT")
            nc.scalar.activation(aT[:], ap_ps[:], AF.Gelu, bias=afib_sb[:])

            # adapter up-projection + residual -> attn2T [H, Q]
            a2t = [
                big23.tile([P, Q], F16, tag=f"a2t{m}", name=f"a2t{m}")
                for m in range(KO)
            ]
            for m in range(KO):
                pp = ps.tile([P, Q], F32, tag="ps512")
                nc.tensor.matmul(
                    pp[:],
                    r(aset_sb[:, m * P:(m + 1) * P]),
                    r(aT[:]),
                    start=True,
                    stop=True,
                )
                if m % 2 == 0:
                    nc.vector.scalar_tensor_tensor(
                        a2t[m][:],
                        pp[:],
                        aseb_sb[:, m:m + 1],
                        aot[m][:],
                        mybir.AluOpType.add,
                        mybir.AluOpType.add,
                    )
                else:
                    # route through ScalarE (idle here) to unclog VectorE
                    tmp = mid_pool.tile([P, Q], F16, tag="a2tmp")
                    nc.scalar.activation(
                        tmp[:], pp[:], AF.Identity, bias=aseb_sb[:, m:m + 1]
                    )
                    nc.vector.tensor_add(a2t[m][:], tmp[:], aot[m][:])

            # FFN
            yts = [
                big23.tile([P, Q], F16, tag=f"yt{m}", name=f"yt{m}")
                for m in range(KO)
            ]
            with ExitStack() as ffn:
                ypool = ffn.enter_context(
                    tc.tile_pool(name="yp", bufs=1, space="PSUM")
                )
                wi_pool = ffn.enter_context(tc.tile_pool(name="wi", bufs=2))
                wo_pool = ffn.enter_context(tc.tile_pool(name="wo", bufs=3))
                it_pool = ffn.enter_context(tc.tile_pool(name="it", bufs=3))

                ytiles = [
                    ypool.tile([P, Q], F32, tag=f"y{m}", name=f"y{m}")
                    for m in range(KO)
                ]
                def emit_ffn2(k, itw, wo_t):
                    for m in range(KO):
                        nc.tensor.matmul(
                            ytiles[m][:],
                            r(wo_t[:, m * P:(m + 1) * P]),
                            r(itw[:]),
                            start=(k == 0),
                            stop=(k == FFT - 1),
                        )

                # FFN2 for tile k is delayed one tile so PE never waits on
                # the gelu between FFN1 and FFN2
                itp = None
                prev = None
                for kc in range(KO):  # 6 chunks of 512 FF rows
                    wchunk = wi_pool.tile([P, KO, 512], F16, tag="wchunk")
                    nc.sync.dma_start(
                        wchunk[:], part6(wit[:, kc * 512:(kc + 1) * 512])
                    )
                    for kk in range(4):
                        k = kc * 4 + kk
                        ip = ps.tile([P, Q], F32, tag="ps512")
                        for k6 in range(KO):
                            nc.tensor.matmul(
                                ip[:],
                                r(wchunk[:, k6, kk * P:(kk + 1) * P]),
                                r(a2t[k6][:]),
                                start=(k6 == 0),
                                stop=(k6 == KO - 1),
                            )
                        itp = it_pool.tile([P, Q], F16, tag="it")
                        nc.scalar.activation(
                            itp[:], ip[:], AF.Gelu, bias=bi_sb[:, k:k + 1]
                        )
                        wo_t = wo_pool.tile([P, H], F16, tag="wo_t")
                        nc.gpsimd.dma_start(wo_t[:], wot[k * P:(k + 1) * P, :])
                        if prev is not None:
                            emit_ffn2(*prev)
                        prev = (k, itp, wo_t)
                if prev is not None:
                    emit_ffn2(*prev)
                # prefetch the sqrt table while the FFN tail drains (Square
                # is a filler function present in every table set); reading
                # the last gelu output keeps this after every gelu
                junk3 = small.tile([1, 1], F32, tag="junk3")
                nc.scalar.activation(
                    junk3[:], itp[0:1, 0:1], AF.Sqrt, bias=one_sb[:], scale=0.0
                )
                for m in range(KO):
                    if m % 2 == 0:
                        nc.vector.scalar_tensor_tensor(
                            yts[m][:],
                            ytiles[m][:],
                            bo_sb[:, m:m + 1],
                            a2t[m][:],
                            mybir.AluOpType.add,
                            mybir.AluOpType.add,
                        )
                    else:
                        # route through ScalarE to unclog VectorE (same
                        # split as the adapter residual)
                        ytmp = mid_pool.tile([P, Q], F16, tag="a2tmp")
                        nc.scalar.activation(
                            ytmp[:], ytiles[m][:], AF.Identity,
                            bias=bo_sb[:, m:m + 1],
                        )
                        nc.vector.tensor_add(yts[m][:], ytmp[:], a2t[m][:])

            # LayerNorm over H (partition dim across 6 tiles):
            # mean / mean-of-squares via ones-column matmuls
            lnp = s23.enter_context(tc.tile_pool(name="lnp", bufs=1, space="PSUM"))
            mu_ps = lnp.tile([1, Q], F32, tag="mu")
            for m in range(KO):
                nc.tensor.matmul(
                    mu_ps[:], r(ones_col[:]), r(yts[m][:]),
                    start=(m == 0), stop=(m == KO - 1),
                )
            sq_ps = lnp.tile([1, Q], F32, tag="sq")
            for m in range(KO):
                sqt = mid_pool.tile([P, Q], F16, tag="sqt")
                nc.scalar.activation(sqt[:], yts[m][:], AF.Square)
                nc.tensor.matmul(
                    sq_ps[:], r(ones_col[:]), r(sqt[:]),
                    start=(m == 0), stop=(m == KO - 1),
                )

            mu = small.tile([1, Q], F32, tag="mu_sb")
            nc.vector.tensor_scalar_mul(mu[:], mu_ps[:], 1.0 / H)
            mu2h = small.tile([1, Q], F32, tag="mu2")
            nc.vector.tensor_mul(mu2h[:], mu_ps[:], mu[:])   # H * mu^2
            vh = small.tile([1, Q], F32, tag="vh")
            nc.vector.tensor_sub(vh[:], sq_ps[:], mu2h[:])   # H * var
            eps_sb = small.tile([1, 1], F32, tag="eps")
            nc.vector.memset(eps_sb[:], EPS)
            sd = small.tile([1, Q], F32, tag="sd")
            nc.scalar.activation(sd[:], vh[:], AF.Sqrt, bias=eps_sb[:],
                                 scale=1.0 / H)
            rs = small.tile([1, Q], F16, tag="rs")
            nc.vector.reciprocal(rs[:], sd[:])
            nm = small.tile([1, Q], F16, tag="nm")
            nc.vector.scalar_tensor_tensor(
                nm[:], mu[:], -1.0, rs[:],
                mybir.AluOpType.mult, mybir.AluOpType.mult,
            )
            bb23 = s23.enter_context(tc.tile_pool(name="bb23", bufs=1, space="PSUM"))
            Ab = bb23.tile([P, Q], F32, tag="Ab")
            nc.tensor.matmul(Ab[:], r(ones_row[:]), r(rs[:]), start=True, stop=True)
            Mb = bb23.tile([P, Q], F32, tag="Mb")
            nc.tensor.matmul(Mb[:], r(ones_row[:]), r(nm[:]), start=True, stop=True)
            Abs = small.tile([P, Q], F16, tag="Abs")
            nc.scalar.activation(Abs[:], Ab[:], AF.Copy)
            Mbs = small.tile([P, Q], F16, tag="Mbs")
            nc.scalar.activation(Mbs[:], Mb[:], AF.Copy)
            outt_p = part6(outt)
            for m in range(KO):
                # (y - mu)/sd * ln_g + ln_b, all on VectorE in fp16:
                # two tensor-tensor ops plus one fused two-scalar op
                t1 = mid_pool.tile([P, Q], F16, tag="sqt")
                nc.vector.tensor_mul(t1[:], yts[m][:], Abs[:])
                nc.vector.tensor_add(t1[:], t1[:], Mbs[:])
                nc.vector.tensor_scalar(
                    t1[:], t1[:], lng_sb[:, m:m + 1], lnb_sb[:, m:m + 1],
                    mybir.AluOpType.mult, mybir.AluOpType.add,
                )
                if m % 2 == 0:
                    nc.sync.dma_start(outt_p[:, m, :], t1[:])
                else:
                    nc.gpsimd.dma_start(outt_p[:, m, :], t1[:])

    nc.compile()
    return nc


_NC_CACHE = {}


def _get_nc(uniform_mask: bool = True):
    if uniform_mask not in _NC_CACHE:
        _NC_CACHE[uniform_mask] = _build_nc(uniform_mask)
    return _NC_CACHE[uniform_mask]


def make_in_maps(
    hidden_states, attention_mask, wq, bq, wk, bk, wv, bv,
    a_fi_w, a_fi_b, a_se_w, a_se_b, wi, bi, wo, bo, ln_g, ln_b,
):
    f = np.float32
    h16 = np.float16
    ca = np.ascontiguousarray

    def part_bias(v, n):  # [n*128] -> [128, n]
        return ca(np.asarray(v, f).reshape(n, P).T)

    shared = {
        "wqt": ca(np.asarray(wq, h16).T),
        "wkt": ca(np.asarray(wk, h16).T),
        "wvt": ca(np.asarray(wv, h16).T),
        "afit": ca(np.asarray(a_fi_w, h16).T),
        "aset": ca(np.asarray(a_se_w, h16).T),
        "wit": ca(np.asarray(wi, h16).T),
        "wot": ca(np.asarray(wo, h16).T),
    }

    def _consts(mask_b):
        c = np.zeros((P, CC), f)
        # -2.0 guards against fp16 overflow of exp(); cancels in softmax
        c[:, 0:16] = mask_b.reshape(S // P, P).T - 2.0
        c[:, 16:22] = part_bias(bq, KO)
        c[:, 22:28] = part_bias(bv, KO)
        c[:, 28:34] = part_bias(a_se_b, KO)
        c[:, 34:40] = part_bias(bo, KO)
        c[:, 40:46] = part_bias(ln_g, KO)
        c[:, 46:52] = part_bias(ln_b, KO)
        c[:, 52:76] = part_bias(bi, FFT)
        c[0:AD, 76] = np.asarray(a_fi_b, f)
        return c

    hs = np.asarray(hidden_states)
    am = np.asarray(attention_mask, f)
    in_maps = []
    for c in range(NCORES):
        b = c // (NCORES // B)
        q0 = (c % (NCORES // B)) * Q
        m = dict(shared)
        m["xt"] = ca(hs[b].T.astype(h16))
        m["xtq"] = ca(hs[b, q0:q0 + Q].T.astype(h16))
        m["consts"] = _consts(am[b, 0, 0])
        in_maps.append(m)
    return in_maps


def gather_out(results):
    out = np.empty((B, S, H), np.float32)
    for c in range(NCORES):
        b = c // (NCORES // B)
        q0 = (c % (NCORES // B)) * Q
        out[b, q0:q0 + Q, :] = results[c]["outt"].T
    return out


def kernel(**inputs):
    uniform = not np.any(np.asarray(inputs["attention_mask"]))
    nc = _get_nc(uniform)
    in_maps = make_in_maps(**inputs)
    res = run_bass_kernel_spmd(nc, in_maps, core_ids=list(range(NCORES)))
    return gather_out(res.results)
